# revision 5
# baseline (speedup 1.0000x reference)
"""Trainium2 Bass kernel for nn_MultiHeadAttention_84576495993495.

Key observation: the reference module's output einsum is
    out = einsum('bhqk,bhvo->bhvo', attn, v)
which contracts softmax(attn) over BOTH q and k. Every softmax row sums
to 1, so sum_{q,k} attn == S (= 2048) and the whole attention block
collapses to out == S * v. Hence

    reference(x, ...) == ((x @ Wv.T + bv) * S) @ Wp.T + bp
                      ==  x @ M + c
with
    M = S * Wv.T @ Wp.T          (folded on host in fp64, then split)
    c = S * Wp @ bv + bp

(Verified vs the jax reference: rel Frobenius err ~3.6e-7 = fp32 noise.)

Device work: the data-dependent GEMM y = x @ M + c, sharded
data-parallel over the 8192 rows -> 1024 rows per NeuronCore.

Precision strategy: TensorE native fp32 matmul runs at 4 cyc/row (and
measures ~2x worse than that on HW); fp16 runs at 1 cyc/row.  So x and
M are each split into a high + low fp16 pair (x = xh + xl, M = Mh + Ml,
each pair exact to ~2^-22 relative) and the GEMM is computed as three
fp16 passes accumulated in the same fp32 PSUM group:
    y = xh@Mh + xh@Ml + xl@Mh  (+ c)
The dropped xl@Ml term is ~2^-22 relative -- below fp32 round-off for
this problem.  CPU-verified: rel err 3.56e-7, identical to a pure-fp32
evaluation of the same GEMM.

Layout: the TensorE contracts over the partition dim, so the x shard is
fed pre-transposed (host-side layout prep; fp32/fp16 DMA-transpose of
the activation on-device is not worth it here).  Per n-chunk half, the
schedule is k-major across 8 live PSUM banks so the PE only ever waits
for one (x[k], M[k]) tile pair (~384 KB) instead of the whole working
set, and the moving operand (rhs) stays constant across each 8-matmul
inner sweep (measured faster than chaining each bank's accumulation
contiguously: 137.7 vs 160.5 us steady-state).

Measured on HW (8 cores, axon): rel err vs reference 2.554e-07
(absmax 2.2e-3 on a 5.3e+3 scale); steady-state body time ~138 us/core
(For_i loop slope over T in {1, 8193, 16385}); native-fp32 variant of
the same kernel measures ~247 us, float32r ~matches fp16x3 speed but
with rel err 1.25e-4.
"""

import os
from functools import lru_cache

import numpy as np

# Defensive: a previous run crashing mid-execution can leave the NeuronCores
# in an unrecoverable state (NRT_EXEC_UNIT_UNRECOVERABLE); resetting cores at
# NRT init clears it and is harmless otherwise.
os.environ.setdefault("NEURON_RT_RESET_CORES", "1")

import concourse.bass as bass
import concourse.mybir as mybir
import concourse.tile as tile
from concourse import bacc
from concourse.bass_utils import run_bass_kernel_spmd

N_CORES = 8
P = 128
D = 1024                       # model dim (= SLICE_SIZE)
B, S = 4, 2048
R_TOTAL = B * S                # 8192 rows
R_CORE = R_TOTAL // N_CORES    # 1024 rows per core
K_TILES = D // P               # 8
R_TILES = R_CORE // P          # 8
N_CHUNK = 512                  # one PSUM bank / fp32 moving-operand max
N_CHUNKS = D // N_CHUNK        # 2
SCALE = float(S)               # sum over q,k of softmax rows == S

# "fp16x1" (default) | "fp16x3" | "float32" | "float32r"
# fp16x1: single fp16 pass (y = xh@Mh + c). CPU-verified rel err 2.9e-4 vs
# the 2e-2 harness tolerance -- the xl/Ml correction passes buy precision
# (2.5e-7) that this problem does not need, at 3x the PE time.
MM_MODE = os.environ.get("KMM_DTYPE", "fp16x1")


@lru_cache(maxsize=4)
def _build_nc(mode: str, loop_iters: int | None = None, order: str | None = None):
    """loop_iters: when set, wrap the compute body in a tc.For_i hardware
    loop (inputs loaded once) -- used by the benchmark harness to measure
    steady-state per-iteration device time without NTFF profiling."""
    if order is None:
        order = os.environ.get("KMM_ORDER", "kmajor")
    split = mode == "fp16x3"
    mm_dt = (
        mybir.dt.float16
        if mode in ("fp16x3", "fp16x1")
        else getattr(mybir.dt, mode)
    )
    nc = bacc.Bacc(None, target_bir_lowering=False)

    if split:
        x_names, m_names = ["xh", "xl"], ["Mh", "Ml"]
    else:
        x_names, m_names = ["xh"], ["Mh"]
    x_dram = [
        nc.dram_tensor(n, [D, R_CORE], mm_dt, kind="ExternalInput") for n in x_names
    ]
    m_dram = [nc.dram_tensor(n, [D, D], mm_dt, kind="ExternalInput") for n in m_names]
    cb = nc.dram_tensor("cb", [P, D], mybir.dt.float32, kind="ExternalInput")
    y = nc.dram_tensor("y", [R_CORE, D], mybir.dt.float32, kind="ExternalOutput")

    x_t = [t.rearrange("(ko p) r -> p ko r", p=P) for t in x_dram]   # [128, 8, 1024]
    m_t = [t.rearrange("(ko p) n -> p ko n", p=P) for t in m_dram]   # [128, 8, 1024]

    # (x operand, M operand) per accumulation pass; the xl@Ml term is dropped.
    passes = [(0, 0), (0, 1), (1, 0)] if split else [(0, 0)]

    with tile.TileContext(nc) as tc:
        with (
            tc.tile_pool(name="wpool", bufs=1) as wpool,
            tc.tile_pool(name="opool", bufs=4) as opool,
            tc.tile_pool(name="pspool", bufs=8, space="PSUM") as pspool,
        ):
            x_sb = [
                wpool.tile([P, K_TILES, R_CORE], mm_dt, tag=f"x_sb{i}", name=f"x_sb{i}")
                for i in range(len(x_dram))
            ]
            m_sb = [
                wpool.tile([P, K_TILES, D], mm_dt, tag=f"m_sb{i}", name=f"m_sb{i}")
                for i in range(len(m_dram))
            ]
            cb_sb = wpool.tile([P, D], mybir.dt.float32, tag="cb_sb")

            nc.sync.dma_start(cb_sb[:], cb[:])
            # Load in pass-0 consumption order first (xh, Mh), then the
            # low halves; per-k granularity so the PE can chase the stream.
            for i in range(len(x_dram)):
                for k in range(K_TILES):
                    nc.sync.dma_start(x_sb[i][:, k], x_t[i][:, k])
                    for nch in range(N_CHUNKS):
                        nc.sync.dma_start(
                            m_sb[i][:, k, bass.ts(nch, N_CHUNK)],
                            m_t[i][:, k, bass.ts(nch, N_CHUNK)],
                        )

            n_acc = len(passes) * K_TILES

            def emit_tail(r, nch, ps):
                out_sb = opool.tile([P, N_CHUNK], mybir.dt.float32, tag="out_sb")
                nc.vector.tensor_add(
                    out_sb[:], ps[:], cb_sb[:, bass.ts(nch, N_CHUNK)]
                )
                nc.sync.dma_start(
                    y[bass.ts(r, P), bass.ts(nch, N_CHUNK)], out_sb[:]
                )

            def body_kmajor():
                # k-major across 8 live PSUM banks (bank switch every MM)
                for nch in range(N_CHUNKS):
                    groups = [
                        pspool.tile([P, N_CHUNK], mybir.dt.float32, tag="ps", name="ps")
                        for _ in range(R_TILES)
                    ]
                    step = 0
                    for xi, mi in passes:
                        for k in range(K_TILES):
                            for r in range(R_TILES):
                                nc.tensor.matmul(
                                    groups[r][:],
                                    x_sb[xi][:, k, bass.ts(r, P)],
                                    m_sb[mi][:, k, bass.ts(nch, N_CHUNK)],
                                    start=(step == 0),
                                    stop=(step == n_acc - 1),
                                )
                            step += 1
                    for r in range(R_TILES):
                        emit_tail(r, nch, groups[r])

            def body_chain():
                # group-major: each PSUM bank's accumulation chain runs as
                # consecutive MMs (no bank cycling between accumulate steps)
                for nch in range(N_CHUNKS):
                    for r in range(R_TILES):
                        ps = pspool.tile(
                            [P, N_CHUNK], mybir.dt.float32, tag="ps", name="ps"
                        )
                        step = 0
                        for xi, mi in passes:
                            for k in range(K_TILES):
                                nc.tensor.matmul(
                                    ps[:],
                                    x_sb[xi][:, k, bass.ts(r, P)],
                                    m_sb[mi][:, k, bass.ts(nch, N_CHUNK)],
                                    start=(step == 0),
                                    stop=(step == n_acc - 1),
                                )
                                step += 1
                        emit_tail(r, nch, ps)

            body = body_chain if order == "chain" else body_kmajor

            if loop_iters is None:
                body()
            else:
                with tc.For_i(0, loop_iters, 1):
                    body()
    nc.compile()
    return nc


def _host_prep(x, Wv, bv, Wp, bp, mode=None):
    mode = mode or MM_MODE
    X = np.ascontiguousarray(x, dtype=np.float32).reshape(R_TOTAL, D)
    M64 = SCALE * (Wv.T.astype(np.float64) @ Wp.T.astype(np.float64))
    c = (SCALE * (Wp.astype(np.float64) @ bv.astype(np.float64)) + bp).astype(
        np.float32
    )
    cbt = np.ascontiguousarray(np.broadcast_to(c, (P, D)))

    if mode == "fp16x3":
        Mh = M64.astype(np.float16)
        Ml = (M64 - Mh.astype(np.float64)).astype(np.float16)
        m_arrs = {"Mh": Mh, "Ml": Ml}
    elif mode == "fp16x1":
        m_arrs = {"Mh": M64.astype(np.float16)}
    else:
        m_arrs = {"Mh": M64.astype(np.float32)}

    in_maps = []
    for i in range(N_CORES):
        shard_t = np.ascontiguousarray(X[i * R_CORE : (i + 1) * R_CORE].T)
        im = dict(m_arrs)
        im["cb"] = cbt
        if mode == "fp16x3":
            xh = shard_t.astype(np.float16)
            xl = (shard_t - xh.astype(np.float32)).astype(np.float16)
            im["xh"] = xh
            im["xl"] = xl
        elif mode == "fp16x1":
            im["xh"] = shard_t.astype(np.float16)
        else:
            im["xh"] = shard_t
        in_maps.append(im)
    return in_maps


def kernel(x, Wq, bq, Wk, bk, Wv, bv, Wp, bp):
    x, Wv, bv, Wp, bp = (np.asarray(a) for a in (x, Wv, bv, Wp, bp))
    nc = _build_nc(MM_MODE)
    in_maps = _host_prep(x, Wv, bv, Wp, bp)
    res = run_bass_kernel_spmd(nc, in_maps, core_ids=list(range(N_CORES)))
    y = np.concatenate([r["y"] for r in res.results], axis=0)
    return y.reshape(B, S, D)



# revision 6
# speedup vs baseline: 1.0624x; 1.0624x over previous
"""Trainium2 Bass kernel for nn_MultiHeadAttention_84576495993495.

Key observation: the reference module's output einsum is
    out = einsum('bhqk,bhvo->bhvo', attn, v)
which contracts softmax(attn) over BOTH q and k. Every softmax row sums
to 1, so sum_{q,k} attn == S (= 2048) and the whole attention block
collapses to out == S * v. Hence

    reference(x, ...) == ((x @ Wv.T + bv) * S) @ Wp.T + bp
                      ==  x @ M + c
with
    M = S * Wv.T @ Wp.T          (folded on host in fp64, then split)
    c = S * Wp @ bv + bp

(Verified vs the jax reference: rel Frobenius err ~3.6e-7 = fp32 noise.)

Device work: the data-dependent GEMM y = x @ M + c, sharded
data-parallel over the 8192 rows -> 1024 rows per NeuronCore.

Precision strategy: TensorE native fp32 matmul runs at 4 cyc/row (and
measures ~2x worse than that on HW); fp16 runs at 1 cyc/row.  So x and
M are each split into a high + low fp16 pair (x = xh + xl, M = Mh + Ml,
each pair exact to ~2^-22 relative) and the GEMM is computed as three
fp16 passes accumulated in the same fp32 PSUM group:
    y = xh@Mh + xh@Ml + xl@Mh  (+ c)
The dropped xl@Ml term is ~2^-22 relative -- below fp32 round-off for
this problem.  CPU-verified: rel err 3.56e-7, identical to a pure-fp32
evaluation of the same GEMM.

Layout: the TensorE contracts over the partition dim, so the x shard is
fed pre-transposed (host-side layout prep; fp32/fp16 DMA-transpose of
the activation on-device is not worth it here).  Per n-chunk half, the
schedule is k-major across 8 live PSUM banks so the PE only ever waits
for one (x[k], M[k]) tile pair (~384 KB) instead of the whole working
set, and the moving operand (rhs) stays constant across each 8-matmul
inner sweep (measured faster than chaining each bank's accumulation
contiguously: 137.7 vs 160.5 us steady-state).

Measured on HW (8 cores, axon): rel err vs reference 2.554e-07
(absmax 2.2e-3 on a 5.3e+3 scale); steady-state body time ~138 us/core
(For_i loop slope over T in {1, 8193, 16385}); native-fp32 variant of
the same kernel measures ~247 us, float32r ~matches fp16x3 speed but
with rel err 1.25e-4.
"""

import os
from functools import lru_cache

import numpy as np

# Defensive: a previous run crashing mid-execution can leave the NeuronCores
# in an unrecoverable state (NRT_EXEC_UNIT_UNRECOVERABLE); resetting cores at
# NRT init clears it and is harmless otherwise.
os.environ.setdefault("NEURON_RT_RESET_CORES", "1")

import concourse.bass as bass
import concourse.mybir as mybir
import concourse.tile as tile
from concourse import bacc
from concourse.bass_utils import run_bass_kernel_spmd

N_CORES = 8
P = 128
D = 1024                       # model dim (= SLICE_SIZE)
B, S = 4, 2048
R_TOTAL = B * S                # 8192 rows
R_CORE = R_TOTAL // N_CORES    # 1024 rows per core
K_TILES = D // P               # 8
R_TILES = R_CORE // P          # 8
N_CHUNK = 512                  # one PSUM bank / fp32 moving-operand max
N_CHUNKS = D // N_CHUNK        # 2
SCALE = float(S)               # sum over q,k of softmax rows == S

# "fp16x1" (default) | "fp16x3" | "float32" | "float32r"
# fp16x1: single fp16 pass (y = xh@Mh + c). CPU-verified rel err 2.9e-4 vs
# the 2e-2 harness tolerance -- the xl/Ml correction passes buy precision
# (2.5e-7) that this problem does not need, at 3x the PE time.
MM_MODE = os.environ.get("KMM_DTYPE", "fp16x1")


@lru_cache(maxsize=4)
def _build_nc(mode: str, loop_iters: int | None = None, order: str | None = None):
    """loop_iters: when set, wrap the compute body in a tc.For_i hardware
    loop (inputs loaded once) -- used by the benchmark harness to measure
    steady-state per-iteration device time without NTFF profiling."""
    if order is None:
        order = os.environ.get("KMM_ORDER", "kmajor")
    split = mode == "fp16x3"
    mm_dt = (
        mybir.dt.float16
        if mode in ("fp16x3", "fp16x1")
        else getattr(mybir.dt, mode)
    )
    nc = bacc.Bacc(None, target_bir_lowering=False)

    if split:
        x_names, m_names = ["xh", "xl"], ["Mh", "Ml"]
    else:
        x_names, m_names = ["xh"], ["Mh"]
    x_dram = [
        nc.dram_tensor(n, [D, R_CORE], mm_dt, kind="ExternalInput") for n in x_names
    ]
    m_dram = [nc.dram_tensor(n, [D, D], mm_dt, kind="ExternalInput") for n in m_names]
    cb = nc.dram_tensor("cb", [P, D], mybir.dt.float32, kind="ExternalInput")
    y = nc.dram_tensor("y", [R_CORE, D], mybir.dt.float32, kind="ExternalOutput")

    x_t = [t.rearrange("(ko p) r -> p ko r", p=P) for t in x_dram]   # [128, 8, 1024]
    m_t = [t.rearrange("(ko p) n -> p ko n", p=P) for t in m_dram]   # [128, 8, 1024]

    # (x operand, M operand) per accumulation pass; the xl@Ml term is dropped.
    passes = [(0, 0), (0, 1), (1, 0)] if split else [(0, 0)]

    with tile.TileContext(nc) as tc:
        with (
            tc.tile_pool(name="wpool", bufs=1) as wpool,
            tc.tile_pool(name="opool", bufs=16) as opool,
            tc.tile_pool(name="pspool", bufs=8, space="PSUM") as pspool,
        ):
            x_sb = [
                wpool.tile([P, K_TILES, R_CORE], mm_dt, tag=f"x_sb{i}", name=f"x_sb{i}")
                for i in range(len(x_dram))
            ]
            m_sb = [
                wpool.tile([P, K_TILES, D], mm_dt, tag=f"m_sb{i}", name=f"m_sb{i}")
                for i in range(len(m_dram))
            ]
            cb_sb = wpool.tile([P, D], mybir.dt.float32, tag="cb_sb")

            nc.sync.dma_start(cb_sb[:], cb[:])
            # Load in pass-0 consumption order first (xh, Mh), then the
            # low halves; per-k granularity so the PE can chase the stream.
            for i in range(len(x_dram)):
                for k in range(K_TILES):
                    nc.sync.dma_start(x_sb[i][:, k], x_t[i][:, k])
                    for nch in range(N_CHUNKS):
                        nc.sync.dma_start(
                            m_sb[i][:, k, bass.ts(nch, N_CHUNK)],
                            m_t[i][:, k, bass.ts(nch, N_CHUNK)],
                        )

            n_acc = len(passes) * K_TILES

            def emit_tail(r, nch, ps):
                out_sb = opool.tile([P, N_CHUNK], mybir.dt.float32, tag="out_sb")
                nc.vector.tensor_add(
                    out_sb[:], ps[:], cb_sb[:, bass.ts(nch, N_CHUNK)]
                )
                nc.sync.dma_start(
                    y[bass.ts(r, P), bass.ts(nch, N_CHUNK)], out_sb[:]
                )

            def body_kmajor():
                # k-major across 8 live PSUM banks (bank switch every MM)
                for nch in range(N_CHUNKS):
                    groups = [
                        pspool.tile([P, N_CHUNK], mybir.dt.float32, tag="ps", name="ps")
                        for _ in range(R_TILES)
                    ]
                    step = 0
                    for xi, mi in passes:
                        for k in range(K_TILES):
                            for r in range(R_TILES):
                                nc.tensor.matmul(
                                    groups[r][:],
                                    x_sb[xi][:, k, bass.ts(r, P)],
                                    m_sb[mi][:, k, bass.ts(nch, N_CHUNK)],
                                    start=(step == 0),
                                    stop=(step == n_acc - 1),
                                )
                            step += 1
                    for r in range(R_TILES):
                        emit_tail(r, nch, groups[r])

            def body_chain():
                # group-major: each PSUM bank's accumulation chain runs as
                # consecutive MMs (no bank cycling between accumulate steps)
                for nch in range(N_CHUNKS):
                    for r in range(R_TILES):
                        ps = pspool.tile(
                            [P, N_CHUNK], mybir.dt.float32, tag="ps", name="ps"
                        )
                        step = 0
                        for xi, mi in passes:
                            for k in range(K_TILES):
                                nc.tensor.matmul(
                                    ps[:],
                                    x_sb[xi][:, k, bass.ts(r, P)],
                                    m_sb[mi][:, k, bass.ts(nch, N_CHUNK)],
                                    start=(step == 0),
                                    stop=(step == n_acc - 1),
                                )
                                step += 1
                        emit_tail(r, nch, ps)

            body = body_chain if order == "chain" else body_kmajor

            if loop_iters is None:
                body()
            else:
                with tc.For_i(0, loop_iters, 1):
                    body()
    nc.compile()
    return nc


def _host_prep(x, Wv, bv, Wp, bp, mode=None):
    mode = mode or MM_MODE
    X = np.ascontiguousarray(x, dtype=np.float32).reshape(R_TOTAL, D)
    M64 = SCALE * (Wv.T.astype(np.float64) @ Wp.T.astype(np.float64))
    c = (SCALE * (Wp.astype(np.float64) @ bv.astype(np.float64)) + bp).astype(
        np.float32
    )
    cbt = np.ascontiguousarray(np.broadcast_to(c, (P, D)))

    if mode == "fp16x3":
        Mh = M64.astype(np.float16)
        Ml = (M64 - Mh.astype(np.float64)).astype(np.float16)
        m_arrs = {"Mh": Mh, "Ml": Ml}
    elif mode == "fp16x1":
        m_arrs = {"Mh": M64.astype(np.float16)}
    else:
        m_arrs = {"Mh": M64.astype(np.float32)}

    in_maps = []
    for i in range(N_CORES):
        shard_t = np.ascontiguousarray(X[i * R_CORE : (i + 1) * R_CORE].T)
        im = dict(m_arrs)
        im["cb"] = cbt
        if mode == "fp16x3":
            xh = shard_t.astype(np.float16)
            xl = (shard_t - xh.astype(np.float32)).astype(np.float16)
            im["xh"] = xh
            im["xl"] = xl
        elif mode == "fp16x1":
            im["xh"] = shard_t.astype(np.float16)
        else:
            im["xh"] = shard_t
        in_maps.append(im)
    return in_maps


def kernel(x, Wq, bq, Wk, bk, Wv, bv, Wp, bp):
    x, Wv, bv, Wp, bp = (np.asarray(a) for a in (x, Wv, bv, Wp, bp))
    nc = _build_nc(MM_MODE)
    in_maps = _host_prep(x, Wv, bv, Wp, bp)
    res = run_bass_kernel_spmd(nc, in_maps, core_ids=list(range(N_CORES)))
    y = np.concatenate([r["y"] for r in res.results], axis=0)
    return y.reshape(B, S, D)



# revision 7
# speedup vs baseline: 1.1243x; 1.0583x over previous
"""Trainium2 Bass kernel for nn_MultiHeadAttention_84576495993495.

Key observation: the reference module's output einsum is
    out = einsum('bhqk,bhvo->bhvo', attn, v)
which contracts softmax(attn) over BOTH q and k. Every softmax row sums
to 1, so sum_{q,k} attn == S (= 2048) and the whole attention block
collapses to out == S * v. Hence

    reference(x, ...) == ((x @ Wv.T + bv) * S) @ Wp.T + bp
                      ==  x @ M + c
with
    M = S * Wv.T @ Wp.T          (folded on host in fp64, then split)
    c = S * Wp @ bv + bp

(Verified vs the jax reference: rel Frobenius err ~3.6e-7 = fp32 noise.)

Device work: the data-dependent GEMM y = x @ M + c, sharded
data-parallel over the 8192 rows -> 1024 rows per NeuronCore.

Precision strategy: the harness tolerance is 2e-2; a SINGLE fp16 pass
(x, M both rounded to fp16, fp32 PSUM accumulate) gives rel err 2.5e-4
-- 80x inside tolerance -- at 1/3 the PE time of the fp16x3 split
scheme (which reaches 2.5e-7 but this problem does not need it).
The output is stored fp16 (adds ~2.8e-4 quantization, still ~50x
inside tolerance) which halves the output DMA traffic and doubles the
DVE tail write rate; the host upcasts to fp32.

Schedule (per 512-col n-chunk): k-major accumulation staggered over
two groups of 4 PSUM banks, so each group's 32 matmuls overlap the
OTHER group's tails (bias add on DVE + store). Tails alternate between
the SP and Activation HWDGE rings so output DMA never serializes
behind one descriptor queue. Microbenchmarked on HW (single core,
For_i steady state): rgroup order with tails+DMA = 35.4us vs 43.1us
for the flat k-major order; pure GEMM floor is ~38us (fp16 streams at
~0.60 ns/row sustained, i.e. ~1.7 GHz effective, for every operand
pattern tried; fp8 only reaches 0.85x of that per byte, so split-fp8
schemes lose).

Measured on HW (8 cores, axon, steady-state For_i slope over
T in {1, 16385, 32769}): see test.py output.
"""

import os
from functools import lru_cache

import numpy as np

# Defensive: a previous run crashing mid-execution can leave the NeuronCores
# in an unrecoverable state (NRT_EXEC_UNIT_UNRECOVERABLE); resetting cores at
# NRT init clears it and is harmless otherwise.
os.environ.setdefault("NEURON_RT_RESET_CORES", "1")

import concourse.bass as bass
import concourse.mybir as mybir
import concourse.tile as tile
from concourse import bacc
from concourse.bass_utils import run_bass_kernel_spmd

N_CORES = 8
P = 128
D = 1024                       # model dim (= SLICE_SIZE)
B, S = 4, 2048
R_TOTAL = B * S                # 8192 rows
R_CORE = R_TOTAL // N_CORES    # 1024 rows per core
K_TILES = D // P               # 8
R_TILES = R_CORE // P          # 8
N_CHUNK = 512                  # one PSUM bank of fp32
N_CHUNKS = D // N_CHUNK        # 2
R_GROUP = 4                    # PSUM banks per stagger group
SCALE = float(S)               # sum over q,k of softmax rows == S

# "fp16x1" (default) | "bf16x1" | "fp16x3" | "float32" | "float32r"
MM_MODE = os.environ.get("KMM_DTYPE", "fp16x1")
# output dtype: "f16" (default; host upcasts) | "f32"
OUT_MODE = os.environ.get("KMM_OUT", "f16")


def _mm_dt(mode):
    if mode in ("fp16x3", "fp16x1"):
        return mybir.dt.float16
    if mode == "bf16x1":
        return mybir.dt.bfloat16
    return getattr(mybir.dt, mode)


@lru_cache(maxsize=4)
def _build_nc(mode: str, loop_iters: int | None = None, order: str | None = None):
    """loop_iters: when set, wrap the compute body in a tc.For_i hardware
    loop (inputs loaded once) -- used by the benchmark harness to measure
    steady-state per-iteration device time without NTFF profiling."""
    if order is None:
        order = os.environ.get("KMM_ORDER", "rgroup")
    split = mode == "fp16x3"
    mm_dt = _mm_dt(mode)
    out_dt = mybir.dt.float16 if OUT_MODE == "f16" else mybir.dt.float32
    nc = bacc.Bacc(None, target_bir_lowering=False)

    if split:
        x_names, m_names = ["xh", "xl"], ["Mh", "Ml"]
    else:
        x_names, m_names = ["xh"], ["Mh"]
    x_dram = [
        nc.dram_tensor(n, [D, R_CORE], mm_dt, kind="ExternalInput") for n in x_names
    ]
    m_dram = [nc.dram_tensor(n, [D, D], mm_dt, kind="ExternalInput") for n in m_names]
    cb = nc.dram_tensor("cb", [P, D], mybir.dt.float32, kind="ExternalInput")
    y = nc.dram_tensor("y", [R_CORE, D], out_dt, kind="ExternalOutput")

    x_t = [t.rearrange("(ko p) r -> p ko r", p=P) for t in x_dram]   # [128, 8, 1024]
    m_t = [t.rearrange("(ko p) n -> p ko n", p=P) for t in m_dram]   # [128, 8, 1024]

    # (x operand, M operand) per accumulation pass; the xl@Ml term is dropped.
    passes = [(0, 0), (0, 1), (1, 0)] if split else [(0, 0)]

    with tile.TileContext(nc) as tc:
        with (
            tc.tile_pool(name="wpool", bufs=1) as wpool,
            tc.tile_pool(name="opool", bufs=16) as opool,
            tc.tile_pool(name="pspool", bufs=8, space="PSUM") as pspool,
        ):
            x_sb = [
                wpool.tile([P, K_TILES, R_CORE], mm_dt, tag=f"x_sb{i}", name=f"x_sb{i}")
                for i in range(len(x_dram))
            ]
            m_sb = [
                wpool.tile([P, K_TILES, D], mm_dt, tag=f"m_sb{i}", name=f"m_sb{i}")
                for i in range(len(m_dram))
            ]
            cb_sb = wpool.tile([P, D], mybir.dt.float32, tag="cb_sb")

            nc.sync.dma_start(cb_sb[:], cb[:])
            # Load in consumption order, alternating the two HWDGE rings.
            for i in range(len(x_dram)):
                for k in range(K_TILES):
                    nc.sync.dma_start(x_sb[i][:, k], x_t[i][:, k])
                    for nch in range(N_CHUNKS):
                        nc.scalar.dma_start(
                            m_sb[i][:, k, bass.ts(nch, N_CHUNK)],
                            m_t[i][:, k, bass.ts(nch, N_CHUNK)],
                        )

            n_acc = len(passes) * K_TILES

            def emit_tail(r, nch, ps):
                out_sb = opool.tile([P, N_CHUNK], out_dt, tag="out_sb")
                nc.vector.tensor_add(
                    out_sb[:], ps[:], cb_sb[:, bass.ts(nch, N_CHUNK)]
                )
                eng = nc.sync if r % 2 == 0 else nc.scalar
                eng.dma_start(
                    y[bass.ts(r, P), bass.ts(nch, N_CHUNK)], out_sb[:]
                )

            def body_rgroup():
                # k-major within staggered groups of R_GROUP banks: each
                # group's tails overlap the next group's matmuls.
                for nch in range(N_CHUNKS):
                    groups = [
                        pspool.tile([P, N_CHUNK], mybir.dt.float32, tag="ps", name="ps")
                        for _ in range(R_TILES)
                    ]
                    for g0 in range(0, R_TILES, R_GROUP):
                        step = 0
                        for xi, mi in passes:
                            for k in range(K_TILES):
                                for r in range(g0, g0 + R_GROUP):
                                    nc.tensor.matmul(
                                        groups[r][:],
                                        x_sb[xi][:, k, bass.ts(r, P)],
                                        m_sb[mi][:, k, bass.ts(nch, N_CHUNK)],
                                        start=(step == 0),
                                        stop=(step == n_acc - 1),
                                    )
                                step += 1
                        for r in range(g0, g0 + R_GROUP):
                            emit_tail(r, nch, groups[r])

            def body_kmajor():
                # k-major across 8 live PSUM banks (bank switch every MM)
                for nch in range(N_CHUNKS):
                    groups = [
                        pspool.tile([P, N_CHUNK], mybir.dt.float32, tag="ps", name="ps")
                        for _ in range(R_TILES)
                    ]
                    step = 0
                    for xi, mi in passes:
                        for k in range(K_TILES):
                            for r in range(R_TILES):
                                nc.tensor.matmul(
                                    groups[r][:],
                                    x_sb[xi][:, k, bass.ts(r, P)],
                                    m_sb[mi][:, k, bass.ts(nch, N_CHUNK)],
                                    start=(step == 0),
                                    stop=(step == n_acc - 1),
                                )
                            step += 1
                    for r in range(R_TILES):
                        emit_tail(r, nch, groups[r])

            body = body_rgroup if order == "rgroup" else body_kmajor

            if loop_iters is None:
                body()
            else:
                with tc.For_i(0, loop_iters, 1):
                    body()
    nc.compile()
    return nc


def _np_dt(mode):
    if mode in ("fp16x3", "fp16x1"):
        return np.float16
    if mode == "bf16x1":
        import ml_dtypes

        return ml_dtypes.bfloat16
    return np.float32


def _host_prep(x, Wv, bv, Wp, bp, mode=None):
    mode = mode or MM_MODE
    np_dt = _np_dt(mode)
    X = np.ascontiguousarray(x, dtype=np.float32).reshape(R_TOTAL, D)
    M64 = SCALE * (Wv.T.astype(np.float64) @ Wp.T.astype(np.float64))
    c = (SCALE * (Wp.astype(np.float64) @ bv.astype(np.float64)) + bp).astype(
        np.float32
    )
    cbt = np.ascontiguousarray(np.broadcast_to(c, (P, D)))

    if mode == "fp16x3":
        Mh = M64.astype(np.float16)
        Ml = (M64 - Mh.astype(np.float64)).astype(np.float16)
        m_arrs = {"Mh": Mh, "Ml": Ml}
    else:
        m_arrs = {"Mh": M64.astype(np_dt)}

    in_maps = []
    for i in range(N_CORES):
        shard_t = np.ascontiguousarray(X[i * R_CORE : (i + 1) * R_CORE].T)
        im = dict(m_arrs)
        im["cb"] = cbt
        if mode == "fp16x3":
            xh = shard_t.astype(np.float16)
            xl = (shard_t - xh.astype(np.float32)).astype(np.float16)
            im["xh"] = xh
            im["xl"] = xl
        else:
            im["xh"] = shard_t.astype(np_dt)
        in_maps.append(im)
    return in_maps


def kernel(x, Wq, bq, Wk, bk, Wv, bv, Wp, bp):
    x, Wv, bv, Wp, bp = (np.asarray(a) for a in (x, Wv, bv, Wp, bp))
    nc = _build_nc(MM_MODE)
    in_maps = _host_prep(x, Wv, bv, Wp, bp)
    res = run_bass_kernel_spmd(nc, in_maps, core_ids=list(range(N_CORES)))
    y = np.concatenate(
        [np.asarray(r["y"], dtype=np.float32) for r in res.results], axis=0
    )
    return y.reshape(B, S, D)


# revision 15
# speedup vs baseline: 1.2255x; 1.0900x over previous
"""Trainium2 Bass kernel for nn_MultiHeadAttention_84576495993495.

Key observation: the reference module's output einsum is
    out = einsum('bhqk,bhvo->bhvo', attn, v)
which contracts softmax(attn) over BOTH q and k. Every softmax row sums
to 1, so sum_{q,k} attn == S (= 2048) and the whole attention block
collapses to out == S * v. Hence

    reference(x, ...) == ((x @ Wv.T + bv) * S) @ Wp.T + bp
                      ==  x @ M + c
with
    M = S * Wv.T @ Wp.T          (folded on host in fp64, then split)
    c = S * Wp @ bv + bp

(Verified vs the jax reference: rel Frobenius err ~3.6e-7 = fp32 noise.)

Device work: the data-dependent GEMM y = x @ M + c, sharded
data-parallel over the 8192 rows -> 1024 rows per NeuronCore.

Precision strategy: the harness tolerance is 2e-2; a SINGLE fp16 pass
(x, M both rounded to fp16, fp32 PSUM accumulate) gives rel err 2.5e-4
-- 80x inside tolerance -- at 1/3 the PE time of the fp16x3 split
scheme (which reaches 2.5e-7 but this problem does not need it).
The output is stored fp16 (adds ~2.8e-4 quantization, still ~50x
inside tolerance) which halves the output DMA traffic and doubles the
DVE tail write rate; the host upcasts to fp32.

Schedule (per 512-col n-chunk): k-major accumulation staggered over
two groups of 4 PSUM banks, so each group's 32 matmuls overlap the
OTHER group's tails (bias add on DVE + store). Tails alternate between
the SP and Activation HWDGE rings so output DMA never serializes
behind one descriptor queue. Microbenchmarked on HW (single core,
For_i steady state): rgroup order with tails+DMA = 35.4us vs 43.1us
for the flat k-major order; pure GEMM floor is ~38us (fp16 streams at
~0.60 ns/row sustained, i.e. ~1.7 GHz effective, for every operand
pattern tried; fp8 only reaches 0.85x of that per byte, so split-fp8
schemes lose).

Measured on HW (8 cores, axon, steady-state For_i slope over
T in {1, 16385, 32769}): see test.py output.
"""

import os
from functools import lru_cache

import numpy as np

# Defensive: a previous run crashing mid-execution can leave the NeuronCores
# in an unrecoverable state (NRT_EXEC_UNIT_UNRECOVERABLE); resetting cores at
# NRT init clears it and is harmless otherwise.
os.environ.setdefault("NEURON_RT_RESET_CORES", "1")

import concourse.bass as bass
import concourse.mybir as mybir
import concourse.tile as tile
from concourse import bacc
from concourse.bass_utils import run_bass_kernel_spmd

N_CORES = 8
P = 128
D = 1024                       # model dim (= SLICE_SIZE)
B, S = 4, 2048
R_TOTAL = B * S                # 8192 rows
R_CORE = R_TOTAL // N_CORES    # 1024 rows per core
K_TILES = D // P               # 8
R_TILES = R_CORE // P          # 8
N_CHUNK = int(os.environ.get("KMM_NCHUNK", "512"))  # PSUM cols per group
N_CHUNKS = D // N_CHUNK
PS_BUFS = (8 * 2048) // (N_CHUNK * 4)  # PSUM accumulators that fit
R_GROUP = int(os.environ.get("KMM_RGROUP", "4"))  # banks per stagger group
SCALE = float(S)               # sum over q,k of softmax rows == S

# "bf16x1" (default) | "fp16x1" | "fp16x3" | "float32" | "float32r"
MM_MODE = os.environ.get("KMM_DTYPE", "bf16x1")
# output dtype: "f16" (default; host upcasts) | "f32"
OUT_MODE = os.environ.get("KMM_OUT", "f16")
# tail: "add" (DVE adds bias, device-complete) | "copy" (DVE copy, bias
# added on host during upcast) | "psum" (DMA directly from PSUM, bias on
# host; no DVE in the chain)
TAIL_MODE = os.environ.get("KMM_TAIL", "add")


def _mm_dt(mode):
    if mode in ("fp16x3", "fp16x1"):
        return mybir.dt.float16
    if mode == "bf16x1":
        return mybir.dt.bfloat16
    return getattr(mybir.dt, mode)


@lru_cache(maxsize=4)
def _build_nc(mode: str, loop_iters: int | None = None, order: str | None = None):
    """loop_iters: when set, wrap the compute body in a tc.For_i hardware
    loop (inputs loaded once) -- used by the benchmark harness to measure
    steady-state per-iteration device time without NTFF profiling."""
    if order is None:
        order = os.environ.get("KMM_ORDER", "rgroup")
    split = mode == "fp16x3"
    mm_dt = _mm_dt(mode)
    out_dt = mybir.dt.float16 if OUT_MODE == "f16" else mybir.dt.float32
    if TAIL_MODE == "psum":
        out_dt = mybir.dt.float32  # DMA cannot convert dtypes on HWDGE
    nc = bacc.Bacc(None, target_bir_lowering=False)

    if split:
        x_names, m_names = ["xh", "xl"], ["Mh", "Ml"]
    else:
        x_names, m_names = ["xh"], ["Mh"]
    x_dram = [
        nc.dram_tensor(n, [D, R_CORE], mm_dt, kind="ExternalInput") for n in x_names
    ]
    m_dram = [nc.dram_tensor(n, [D, D], mm_dt, kind="ExternalInput") for n in m_names]
    cb = nc.dram_tensor("cb", [P, D], mybir.dt.float32, kind="ExternalInput")
    y = nc.dram_tensor("y", [R_CORE, D], out_dt, kind="ExternalOutput")

    x_t = [t.rearrange("(ko p) r -> p ko r", p=P) for t in x_dram]   # [128, 8, 1024]
    m_t = [t.rearrange("(ko p) n -> p ko n", p=P) for t in m_dram]   # [128, 8, 1024]

    # (x operand, M operand) per accumulation pass; the xl@Ml term is dropped.
    passes = [(0, 0), (0, 1), (1, 0)] if split else [(0, 0)]

    with tile.TileContext(nc) as tc:
        with (
            tc.tile_pool(name="wpool", bufs=1) as wpool,
            tc.tile_pool(name="opool", bufs=16) as opool,
            tc.tile_pool(name="pspool", bufs=PS_BUFS, space="PSUM") as pspool,
        ):
            x_sb = [
                wpool.tile([P, K_TILES, R_CORE], mm_dt, tag=f"x_sb{i}", name=f"x_sb{i}")
                for i in range(len(x_dram))
            ]
            m_sb = [
                wpool.tile([P, K_TILES, D], mm_dt, tag=f"m_sb{i}", name=f"m_sb{i}")
                for i in range(len(m_dram))
            ]
            cb_sb = wpool.tile([P, D], mybir.dt.float32, tag="cb_sb")

            if TAIL_MODE == "add":
                nc.sync.dma_start(cb_sb[:], cb[:])
            # Load in consumption order, alternating the two HWDGE rings.
            for i in range(len(x_dram)):
                for k in range(K_TILES):
                    nc.sync.dma_start(x_sb[i][:, k], x_t[i][:, k])
                    for nch in range(N_CHUNKS):
                        nc.scalar.dma_start(
                            m_sb[i][:, k, bass.ts(nch, N_CHUNK)],
                            m_t[i][:, k, bass.ts(nch, N_CHUNK)],
                        )

            n_acc = len(passes) * K_TILES

            def emit_tail(r, nch, ps):
                eng = nc.sync if r % 2 == 0 else nc.scalar
                if TAIL_MODE == "psum":
                    eng.dma_start(
                        y[bass.ts(r, P), bass.ts(nch, N_CHUNK)], ps[:]
                    )
                    return
                out_sb = opool.tile([P, N_CHUNK], out_dt, tag="out_sb")
                if TAIL_MODE == "copy":
                    nc.vector.tensor_copy(out_sb[:], ps[:])
                else:
                    nc.vector.tensor_add(
                        out_sb[:], ps[:], cb_sb[:, bass.ts(nch, N_CHUNK)]
                    )
                eng.dma_start(
                    y[bass.ts(r, P), bass.ts(nch, N_CHUNK)], out_sb[:]
                )

            def body_rgroup():
                # k-major within staggered groups of R_GROUP banks: each
                # group's tails overlap the next group's matmuls.
                for nch in range(N_CHUNKS):
                    groups = [
                        pspool.tile([P, N_CHUNK], mybir.dt.float32, tag="ps", name="ps")
                        for _ in range(R_TILES)
                    ]
                    for g0 in range(0, R_TILES, R_GROUP):
                        step = 0
                        for xi, mi in passes:
                            for k in range(K_TILES):
                                for r in range(g0, g0 + R_GROUP):
                                    nc.tensor.matmul(
                                        groups[r][:],
                                        x_sb[xi][:, k, bass.ts(r, P)],
                                        m_sb[mi][:, k, bass.ts(nch, N_CHUNK)],
                                        start=(step == 0),
                                        stop=(step == n_acc - 1),
                                    )
                                step += 1
                        for r in range(g0, g0 + R_GROUP):
                            emit_tail(r, nch, groups[r])

            def body_kmajor():
                # k-major across 8 live PSUM banks (bank switch every MM)
                for nch in range(N_CHUNKS):
                    groups = [
                        pspool.tile([P, N_CHUNK], mybir.dt.float32, tag="ps", name="ps")
                        for _ in range(R_TILES)
                    ]
                    step = 0
                    for xi, mi in passes:
                        for k in range(K_TILES):
                            for r in range(R_TILES):
                                nc.tensor.matmul(
                                    groups[r][:],
                                    x_sb[xi][:, k, bass.ts(r, P)],
                                    m_sb[mi][:, k, bass.ts(nch, N_CHUNK)],
                                    start=(step == 0),
                                    stop=(step == n_acc - 1),
                                )
                            step += 1
                    for r in range(R_TILES):
                        emit_tail(r, nch, groups[r])

            body = body_rgroup if order == "rgroup" else body_kmajor

            if loop_iters is None:
                body()
            else:
                with tc.For_i(0, loop_iters, 1):
                    body()
    nc.compile()
    return nc


def _np_dt(mode):
    if mode in ("fp16x3", "fp16x1"):
        return np.float16
    if mode == "bf16x1":
        import ml_dtypes

        return ml_dtypes.bfloat16
    return np.float32


def _host_prep(x, Wv, bv, Wp, bp, mode=None):
    mode = mode or MM_MODE
    np_dt = _np_dt(mode)
    X = np.ascontiguousarray(x, dtype=np.float32).reshape(R_TOTAL, D)
    M64 = SCALE * (Wv.T.astype(np.float64) @ Wp.T.astype(np.float64))
    c = (SCALE * (Wp.astype(np.float64) @ bv.astype(np.float64)) + bp).astype(
        np.float32
    )
    cbt = np.ascontiguousarray(np.broadcast_to(c, (P, D)))

    if mode == "fp16x3":
        Mh = M64.astype(np.float16)
        Ml = (M64 - Mh.astype(np.float64)).astype(np.float16)
        m_arrs = {"Mh": Mh, "Ml": Ml}
    else:
        m_arrs = {"Mh": M64.astype(np_dt)}

    in_maps = []
    for i in range(N_CORES):
        shard_t = np.ascontiguousarray(X[i * R_CORE : (i + 1) * R_CORE].T)
        im = dict(m_arrs)
        im["cb"] = cbt
        if mode == "fp16x3":
            xh = shard_t.astype(np.float16)
            xl = (shard_t - xh.astype(np.float32)).astype(np.float16)
            im["xh"] = xh
            im["xl"] = xl
        else:
            im["xh"] = shard_t.astype(np_dt)
        in_maps.append(im)
    return in_maps


def kernel(x, Wq, bq, Wk, bk, Wv, bv, Wp, bp):
    x, Wv, bv, Wp, bp = (np.asarray(a) for a in (x, Wv, bv, Wp, bp))
    nc = _build_nc(MM_MODE)
    in_maps = _host_prep(x, Wv, bv, Wp, bp)
    res = run_bass_kernel_spmd(nc, in_maps, core_ids=list(range(N_CORES)))
    y = np.concatenate(
        [np.asarray(r["y"], dtype=np.float32) for r in res.results], axis=0
    )
    if TAIL_MODE != "add":  # bias was not applied on device
        c = (
            SCALE * (Wp.astype(np.float64) @ bv.astype(np.float64)) + bp
        ).astype(np.float32)
        y += c
    return y.reshape(B, S, D)
